# revision 1
# baseline (speedup 1.0000x reference)
"""CrossMamba Trainium2 kernel.

Sharding: 8 cores = 4 batch x 2 d_inner halves (uniform SPMD program; the
residual ms/2 and dwconv bias/2 are added on both halves so the host gather is
a plain sum). Device layout is feature-major [feature, token]. The selective
scan runs natively on the DVE via tensor_tensor_scan (state = a*state + b
along the free/time axis), one scan per (d_state n, 128-row d-block), chained
across token chunks via `initial`. PE does all projections (fp32r / bf16),
LN stats (ones-matmul) and per-token row broadcasts (K=1 matmul). ACT does
Square/Softplus/Silu/Exp. GPSIMD takes the y-accumulation adds. The final 3x3
depthwise conv runs in fp16 on zero-padded row bands.
"""
import numpy as np
import ml_dtypes
from contextlib import ExitStack

import concourse.bass as bass
import concourse.bacc as bacc
import concourse.tile as tile
import concourse.mybir as mybir
from concourse.bass_utils import run_bass_kernel_spmd

F32 = mybir.dt.float32
F32R = mybir.dt.float32r
BF16 = mybir.dt.bfloat16
F16 = mybir.dt.float16
AL = mybir.AluOpType
AF = mybir.ActivationFunctionType

DIM = 384
NST = 16
L = 4096
TC = 512
NCH = L // TC
NB = 3              # 128-row blocks in half d_inner / DIM
NBF = 6             # blocks in full d_inner
EPS = 1e-5
NPC = 35
NPF = 10

bf = ml_dtypes.bfloat16


def _f32(x):
    return np.ascontiguousarray(np.asarray(x, dtype=np.float32))


def _bf16(x):
    return np.ascontiguousarray(np.asarray(x, dtype=np.float32).astype(bf))


def make_core_inputs(inp, bi, half):
    sl = slice(half * 384, (half + 1) * 384)
    ms = np.asarray(inp['ms'], np.float32)[bi]
    pan = np.asarray(inp['pan'], np.float32)[bi]
    ln1w = np.asarray(inp['ln1_w'], np.float32); ln1b = np.asarray(inp['ln1_b'], np.float32)
    ln2w = np.asarray(inp['ln2_w'], np.float32); ln2b = np.asarray(inp['ln2_b'], np.float32)
    ln3w = np.asarray(inp['ln3_w'], np.float32); ln3b = np.asarray(inp['ln3_b'], np.float32)
    W_ip = np.asarray(inp['in_proj_W'], np.float32)
    Wx = W_ip[half * 384:(half + 1) * 384] * ln1w[None, :]
    Wz = W_ip[768 + half * 384:768 + (half + 1) * 384] * ln1w[None, :]
    vx = Wx @ ln1b
    vz = Wz @ ln1b
    Wb_f = np.asarray(inp['in_proj_b_W'], np.float32) * ln2w[None, :]
    vb = Wb_f @ ln2b
    Wc_f = np.asarray(inp['in_proj_c_W'], np.float32) * ln3w[None, :]
    vc = Wc_f @ ln3b
    conv_w = np.asarray(inp['conv_w'], np.float32)[sl]
    silu_x_bias = np.asarray(inp['conv_bias'], np.float32)[sl] + vx * conv_w.sum(-1)
    convb_w = np.asarray(inp['conv_b_w'], np.float32)
    silu_b_bias = np.asarray(inp['conv_b_bias'], np.float32) + vb * convb_w.sum(-1)
    convc_w = np.asarray(inp['conv_c_w'], np.float32)
    silu_c_bias = np.asarray(inp['conv_c_bias'], np.float32) + vc * convc_w.sum(-1)
    A = np.exp(np.asarray(inp['A_log'], np.float32))[sl]  # A_pos = -A
    dw_w = np.asarray(inp['dwconv_w'], np.float32)[:, 0].reshape(384, 9)

    ppc = np.zeros((384, NPC), np.float32)
    ppc[:, 0:16] = A
    ppc[:, 16:20] = conv_w
    ppc[:, 20:29] = dw_w
    ppc[:, 29] = silu_x_bias
    ppc[:, 30] = vz
    ppc[:, 31] = -np.asarray(inp['dt_proj_bias'], np.float32)[sl]
    ppc[:, 32] = np.asarray(inp['D'], np.float32)[sl]
    ppc[:, 33] = np.asarray(inp['dwconv_b'], np.float32) * 0.5
    ppc[:, 34] = np.asarray(inp['reduce_b'], np.float32)

    ppf = np.zeros((768, NPF), np.float32)
    ppf[:, 0:4] = convb_w
    ppf[:, 4:8] = convc_w
    ppf[:, 8] = silu_b_bias
    ppf[:, 9] = silu_c_bias

    return {
        'msT': _f32(ms.T),
        'msTh': _bf16(ms.T),
        'panTh': _bf16(pan.T),
        'w_red': _bf16(np.asarray(inp['reduce_W'], np.float32).T),    # [768, 384]
        'w_xz': _bf16(np.concatenate([Wx.T, Wz.T], 1)),               # [384, 768]
        'w_b': _bf16(Wb_f.T),
        'w_c': _bf16(Wc_f.T),
        'w_xp': _bf16(np.asarray(inp['x_proj_W'], np.float32).T),     # [768, 40]
        'w_xpc': _bf16(np.asarray(inp['x_proj_c_W'], np.float32).T),  # [768, 16]
        'w_dt': _bf16(np.asarray(inp['dt_proj_W'], np.float32)[sl].T),  # [24, 384]
        'w_op': _bf16(np.asarray(inp['out_proj_W'], np.float32)[:, sl].T),  # [384, 384]
        'w_ones': _bf16(np.full((128, 1), 1.0 / 384.0)),
        'w_bc1': _f32(np.ones((1, 128))),
        'w_bc1h': _bf16(np.ones((1, 128))),
        'w_sel': _bf16(np.stack([np.tile((np.arange(16) == n)[:, None], (1, 128)) for n in range(16)], 0).transpose(1, 0, 2).reshape(16, 16 * 128)),
        'w_selc': _bf16(-1.0 * np.stack([np.tile((np.arange(16) == n)[:, None], (1, 128)) for n in range(16)], 0).transpose(1, 0, 2).reshape(16, 16 * 128)),
        'ppc': _f32(ppc.reshape(NB, 128, NPC).transpose(1, 0, 2).reshape(128, NB * NPC)),
        'ppf': _f32(ppf.reshape(NBF, 128, NPF).transpose(1, 0, 2).reshape(128, NBF * NPF)),
    }


def r32(ap):
    return ap.bitcast(F32R)


DEBUG = False

def build_nc():
    nc = bacc.Bacc()
    d = {}
    def din(name, shape, dtype=F32):
        d[name] = nc.dram_tensor(name, shape, dtype, kind="ExternalInput")
    din('msT', [DIM, L]); din('msTh', [DIM, L], BF16); din('panTh', [DIM, L], BF16)
    din('w_red', [768, 384], BF16)
    din('w_xz', [384, 768], BF16); din('w_b', [384, 768], BF16); din('w_c', [384, 768], BF16)
    din('w_xp', [768, 40], BF16); din('w_xpc', [768, 16], BF16)
    din('w_dt', [24, 384], BF16); din('w_op', [384, 384], BF16)
    din('w_ones', [128, 1], BF16); din('w_bc1', [1, 128]); din('w_bc1h', [1, 128], BF16)
    din('w_sel', [16, 16 * 128], BF16); din('w_selc', [16, 16 * 128], BF16)
    din('ppc', [128, NB * NPC]); din('ppf', [128, NBF * NPF])
    d['out'] = nc.dram_tensor('out', [DIM, L], F32, kind="ExternalOutput")
    if DEBUG:
        for nm, sh in [('dbg_cc', [128, TC]), ('dbg_xn', [128, TC]), ('dbg_u', [128, TC]),
                       ('dbg_dtv', [128, TC]), ('dbg_dbl', [40, TC]), ('dbg_h', [128, TC]),
                       ('dbg_y', [128, TC]), ('dbg_gf', [128, TC]), ('dbg_s', [1, TC])]:
            d[nm] = nc.dram_tensor(nm, sh, F32, kind="ExternalOutput")
    with tile.TileContext(nc) as tc:
        with ExitStack() as ctx:
            build_kernel(ctx, tc, d)
    nc.compile()
    return nc


def build_kernel(ctx, tc, dram):
    nc = tc.nc
    wpool = ctx.enter_context(tc.tile_pool(name="w", bufs=1))
    persist = ctx.enter_context(tc.tile_pool(name="pers", bufs=1))
    io = ctx.enter_context(tc.tile_pool(name="io", bufs=2))
    big = ctx.enter_context(tc.tile_pool(name="big", bufs=1))     # chunk-lifetime tiles
    tmp = ctx.enter_context(tc.tile_pool(name="tmp", bufs=2))     # short-lived
    pp = ctx.enter_context(tc.tile_pool(name="pp", bufs=2))       # ping-pong chains
    scanp = ctx.enter_context(tc.tile_pool(name="scan", bufs=2))
    ps = ctx.enter_context(tc.tile_pool(name="ps", bufs=4, space="PSUM"))
    ps40 = ctx.enter_context(tc.tile_pool(name="ps40", bufs=2, space="PSUM"))
    psr = ctx.enter_context(tc.tile_pool(name="psr", bufs=2, space="PSUM"))

    def load_w(name, kblocks, mcols, dtype):
        ts = []
        for k in range(kblocks):
            t = wpool.tile([128, mcols], dtype, tag=f"W{name}{k}")
            nc.sync.dma_start(t[:], dram[name][k * 128:(k + 1) * 128, :])
            ts.append(t)
        return ts

    w_red = load_w('w_red', 6, 384, BF16)
    w_xz = load_w('w_xz', 3, 768, BF16)
    w_b = load_w('w_b', 3, 768, BF16)
    w_c = load_w('w_c', 3, 768, BF16)
    w_xp = load_w('w_xp', 6, 40, BF16)
    w_xpc = load_w('w_xpc', 6, 16, BF16)
    w_op = load_w('w_op', 3, 384, BF16)
    w_dt = wpool.tile([24, 384], BF16, tag="Wdt")
    nc.sync.dma_start(w_dt[:], dram['w_dt'][:, :])
    w_ones = wpool.tile([128, 1], BF16, tag="Wones")
    nc.sync.dma_start(w_ones[:], dram['w_ones'][:, :])
    w_bc1 = wpool.tile([1, 128], F32, tag="Wbc1")
    nc.sync.dma_start(w_bc1[:], dram['w_bc1'][:, :])
    w_bc1h = wpool.tile([1, 128], BF16, tag="Wbc1h")
    nc.sync.dma_start(w_bc1h[:], dram['w_bc1h'][:, :])
    w_sel = wpool.tile([16, 16 * 128], BF16, tag="Wsel")
    nc.sync.dma_start(w_sel[:], dram['w_sel'][:, :])
    w_selc = wpool.tile([16, 16 * 128], BF16, tag="Wselc")
    nc.sync.dma_start(w_selc[:], dram['w_selc'][:, :])
    ppc = wpool.tile([128, NB * NPC], F32, tag="ppc")
    nc.sync.dma_start(ppc[:], dram['ppc'][:, :])
    ppf = wpool.tile([128, NBF * NPF], F32, tag="ppf")
    nc.sync.dma_start(ppf[:], dram['ppf'][:, :])
    epsc = wpool.tile([128, 1], F32, tag="epsc")
    nc.vector.memset(epsc[:], EPS)

    def pc(blk, col):
        return ppc[:, blk * NPC + col:blk * NPC + col + 1]

    def pf(blk, col):
        return ppf[:, blk * NPF + col:blk * NPF + col + 1]

    st = persist.tile([128, NST * NB], F32, tag="st")
    gf_full = [persist.tile([128, L], F16, tag=f"gf{b}", name=f"gf{b}") for b in range(NB)]
    hist_x = [persist.tile([128, 4], BF16, tag=f"hx{b}", name=f"hx{b}") for b in range(NB)]
    hist_b = [persist.tile([128, 4], BF16, tag=f"hb{b}", name=f"hb{b}") for b in range(NBF)]
    hist_c = [persist.tile([128, 4], BF16, tag=f"hc{b}", name=f"hc{b}") for b in range(NBF)]
    for t in hist_x + hist_b + hist_c:
        nc.vector.memset(t[:], 0.0)

    def mm_acc(psum, lhsT_tiles, rhs_tiles, mslice, f32r=False):
        nk = len(lhsT_tiles)
        for k in range(nk):
            lt = lhsT_tiles[k][:, mslice]
            rt = rhs_tiles[k][:]
            if f32r:
                lt, rt = r32(lt), r32(rt)
            nc.tensor.matmul(psum[:], lt, rt, start=(k == 0), stop=(k == nk - 1))

    # ================= chunk loop =================
    for c in range(NCH):
        W = slice(c * TC, (c + 1) * TC)
        ms_s, pan_s, msf_s = [], [], []
        for b_ in range(NB):
            t = io.tile([128, TC], BF16, tag=f"ms{b_}")
            nc.sync.dma_start(t[:], dram['msTh'][b_ * 128:(b_ + 1) * 128, W])
            ms_s.append(t)
            t = io.tile([128, TC], BF16, tag=f"pan{b_}")
            nc.sync.dma_start(t[:], dram['panTh'][b_ * 128:(b_ + 1) * 128, W])
            pan_s.append(t)
            t = io.tile([128, TC], F32, tag=f"msf{b_}")
            nc.sync.dma_start(t[:], dram['msT'][b_ * 128:(b_ + 1) * 128, W])
            msf_s.append(t)

        # concat = reduce(ms;pan) + reduce_b
        cc_s = []
        for mb in range(NB):
            p = ps.tile([128, TC], F32, tag="pmm")
            mm_acc(p, w_red, ms_s + pan_s, slice(mb * 128, (mb + 1) * 128))
            t = big.tile([128, TC], BF16, tag=f"cc{mb}")
            nc.vector.tensor_scalar_add(t[:], p[:], pc(mb, 34))
            cc_s.append(t)
        if DEBUG and c == 0:
            nc.sync.dma_start(dram['dbg_cc'][:, :], cc_s[0][:])

        # LN stats: per-tensor [1,TC] rows (PE matmul base-partition must be 0)
        s_rows, m_rows = [], []
        for i, xs in enumerate((ms_s, pan_s, cc_s)):
            p1 = psr.tile([1, TC], F32, tag="pstat")
            for k in range(NB):
                nc.tensor.matmul(p1[:], w_ones[:], xs[k][:],
                                 start=(k == 0), stop=(k == NB - 1))
            mean_i = tmp.tile([1, TC], F32, tag="rowtmp", bufs=4, name=f"mean{i}")
            nc.vector.tensor_copy(mean_i[:], p1[:])
            p2 = psr.tile([1, TC], F32, tag="pstat")
            for k in range(NB):
                sq = tmp.tile([128, TC], BF16, tag="sq")
                nc.gpsimd.tensor_mul(sq[:], xs[k][:], xs[k][:])
                nc.tensor.matmul(p2[:], w_ones[:], sq[:],
                                 start=(k == 0), stop=(k == NB - 1))
            msq_i = tmp.tile([1, TC], F32, tag="rowtmp", bufs=4, name=f"msq{i}")
            nc.vector.tensor_copy(msq_i[:], p2[:])
            sqm_i = tmp.tile([1, TC], F32, tag="rowtmp", bufs=4, name=f"sqm{i}")
            nc.gpsimd.tensor_mul(sqm_i[:], mean_i[:], mean_i[:])
            var_i = tmp.tile([1, TC], F32, tag="rowtmp", bufs=4, name=f"var{i}")
            nc.vector.tensor_sub(var_i[:], msq_i[:], sqm_i[:])
            lv_i = tmp.tile([1, TC], F32, tag="rowtmp", bufs=4, name=f"lv{i}")
            nc.scalar.activation(lv_i[:], var_i[:], AF.Ln, bias=epsc[0:1, :])
            s_i = tmp.tile([1, TC], F32, tag="srow", bufs=2, name=f"s{i}")
            nc.scalar.activation(s_i[:], lv_i[:], AF.Exp, scale=-0.5)
            m_i = tmp.tile([1, TC], F32, tag="mrow", bufs=2, name=f"m{i}")
            nc.vector.tensor_mul(m_i[:], mean_i[:], s_i[:])
            s_rows.append(s_i); m_rows.append(m_i)
        if DEBUG and c == 0:
            nc.sync.dma_start(dram['dbg_s'][:, :], s_rows[0][:])

        # normalize (broadcast via PE, apply on DVE) -> bf16
        xn = {}
        for i, (nm, xs) in enumerate((('ms', ms_s), ('pan', pan_s), ('cc', cc_s))):
            sb = ps.tile([128, TC], F32, tag="pmm")
            nc.tensor.matmul(sb[:], w_bc1[:], s_rows[i][:],
                             start=True, stop=True)
            mb_ = ps.tile([128, TC], F32, tag="pmm")
            nc.tensor.matmul(mb_[:], w_bc1[:], m_rows[i][:],
                             start=True, stop=True)
            outs = []
            for k in range(NB):
                t1 = tmp.tile([128, TC], F32, tag="xnt")
                nc.vector.tensor_mul(t1[:], xs[k][:], sb[:])
                t2 = big.tile([128, TC], BF16, tag=f"xn{nm}{k}")
                nc.vector.tensor_sub(t2[:], t1[:], mb_[:])
                outs.append(t2)
            xn[nm] = outs
        if DEBUG and c == 0:
            nc.gpsimd.dma_start(dram['dbg_xn'][:, :], xn['ms'][0][:])

        def conv_silu(psum, hist, wcol_fn, bias_ap, utag):
            cx = pp.tile([128, TC + 4], BF16, tag="cx")
            nc.vector.tensor_copy(cx[:, 0:4], hist[:])
            nc.vector.tensor_copy(cx[:, 4:4 + TC], psum[:])
            nc.vector.tensor_copy(hist[:], cx[:, TC:TC + 4])
            acc = pp.tile([128, TC], BF16, tag="cacc")
            nc.vector.tensor_scalar_mul(acc[:], cx[:, 1:1 + TC], wcol_fn(0))
            for k in range(1, 4):
                acc2 = pp.tile([128, TC], BF16, tag="cacc")
                nc.vector.scalar_tensor_tensor(acc2[:], cx[:, 1 + k:1 + k + TC],
                                               wcol_fn(k), acc[:], AL.mult, AL.add)
                acc = acc2
            sg = pp.tile([128, TC], BF16, tag="sg")
            nc.scalar.activation(sg[:], acc[:], AF.Sigmoid, bias=bias_ap)
            u = big.tile([128, TC], BF16, tag=utag)
            nc.vector.scalar_tensor_tensor(u[:], acc[:], bias_ap, sg[:],
                                           AL.add, AL.mult)
            return u

        u_s, sz_s, xb_s, xc_s = [], [], [], []
        for mb in range(NB):
            p = ps.tile([128, TC], F32, tag="pmm")
            mm_acc(p, w_xz, xn['ms'], slice(mb * 128, (mb + 1) * 128))
            u_s.append(conv_silu(p, hist_x[mb], lambda k, m=mb: pc(m, 16 + k),
                                 pc(mb, 29), f"u{mb}"))
        for mb in range(NB):
            p = ps.tile([128, TC], F32, tag="pmm")
            mm_acc(p, w_xz, xn['ms'], slice(384 + mb * 128, 384 + (mb + 1) * 128))
            sgz = pp.tile([128, TC], BF16, tag="sg")
            nc.scalar.activation(sgz[:], p[:], AF.Sigmoid, bias=pc(mb, 30))
            t = big.tile([128, TC], BF16, tag=f"sz{mb}")
            nc.vector.scalar_tensor_tensor(t[:], p[:], pc(mb, 30), sgz[:],
                                           AL.add, AL.mult)
            sz_s.append(t)
        for mb in range(NBF):
            p = ps.tile([128, TC], F32, tag="pmm")
            mm_acc(p, w_b, xn['pan'], slice(mb * 128, (mb + 1) * 128))
            xb_s.append(conv_silu(p, hist_b[mb], lambda k, m=mb: pf(m, k),
                                  pf(mb, 8), f"xb{mb}"))
        for mb in range(NBF):
            p = ps.tile([128, TC], F32, tag="pmm")
            mm_acc(p, w_c, xn['cc'], slice(mb * 128, (mb + 1) * 128))
            xc_s.append(conv_silu(p, hist_c[mb], lambda k, m=mb: pf(m, 4 + k),
                                  pf(mb, 9), f"xc{mb}"))

        if DEBUG and c == 0:
            nc.gpsimd.dma_start(dram['dbg_u'][:, :], u_s[0][:])
        # x_proj / x_proj_c
        p = ps40.tile([40, TC], F32, tag="p40")
        mm_acc(p, w_xp, xb_s, slice(0, 40))
        dbls = big.tile([40, TC], BF16, tag="dbls")
        nc.vector.tensor_copy(dbls[:], p[:])
        p = ps40.tile([16, TC], F32, tag="p40")
        mm_acc(p, w_xpc, xc_s, slice(0, 16))
        cms = big.tile([16, TC], BF16, tag="cms")
        nc.vector.tensor_copy(cms[:], p[:])
        bm16 = big.tile([16, TC], BF16, tag="bm16")
        nc.sync.dma_start(bm16[:], dbls[24:40, :])

        # dt / q
        dtv_s, q_s = [], []
        for mb in range(NB):
            p = ps.tile([128, TC], F32, tag="pmm")
            nc.tensor.matmul(p[:], w_dt[:, mb * 128:(mb + 1) * 128],
                             dbls[0:24, :], start=True, stop=True)
            sgd = pp.tile([128, TC], F32, tag="sgd")
            nc.scalar.activation(sgd[:], p[:], AF.Sigmoid, bias=pc(mb, 31),
                                 scale=-1.0)
            dtv = big.tile([128, TC], F32, tag=f"dtv{mb}")
            nc.scalar.activation(dtv[:], sgd[:], AF.Ln)
            dtv_s.append(dtv)      # dtv = ln(sigmoid(-x)) = -dt
            q = big.tile([128, TC], BF16, tag=f"q{mb}")
            nc.vector.tensor_mul(q[:], dtv[:], u_s[mb][:])   # q = -dt*u
            q_s.append(q)

        if DEBUG and c == 0:
            nc.sync.dma_start(dram['dbg_dtv'][:, :], dtv_s[0][:])
            nc.gpsimd.dma_start(dram['dbg_dbl'][:, :], dbls[:, :])
        # ---- scan over d_state ----
        yacc = [None] * NB
        for n in range(NST):
            adt = F32 if n < 4 else BF16
            pb_ = ps.tile([128, TC], F32, tag="pmm")
            nc.tensor.matmul(pb_[:], w_sel[:, n * 128:(n + 1) * 128], bm16[:],
                             start=True, stop=True)
            bb = scanp.tile([128, TC], BF16, tag="bb")
            nc.scalar.copy(bb[:], pb_[:])
            pcb = ps.tile([128, TC], F32, tag="pmm")
            nc.tensor.matmul(pcb[:], w_selc[:, n * 128:(n + 1) * 128], cms[:],
                             start=True, stop=True)
            cb = scanp.tile([128, TC], BF16, tag="cb")
            nc.scalar.copy(cb[:], pcb[:])
            for blk in range(NB):
                a_t = scanp.tile([128, TC], adt, tag="a")
                nc.scalar.activation(a_t[:], dtv_s[blk][:], AF.Exp, scale=pc(blk, n))
                b_t = scanp.tile([128, TC], BF16, tag="b")
                nc.gpsimd.tensor_mul(b_t[:], q_s[blk][:], bb[:])
                h_t = scanp.tile([128, TC], adt, tag="h")
                init = 0.0 if c == 0 else st[:, n * NB + blk:n * NB + blk + 1]
                nc.vector.tensor_tensor_scan(h_t[:], a_t[:], b_t[:], init,
                                             AL.mult, AL.add)
                nc.vector.tensor_copy(st[:, n * NB + blk:n * NB + blk + 1],
                                      h_t[:, TC - 1:TC])
                if DEBUG and c == 0 and n == 0 and blk == 0:
                    nc.gpsimd.dma_start(dram['dbg_h'][:, :], h_t[:])
                p_t = scanp.tile([128, TC], BF16, tag="p")
                nc.vector.tensor_mul(p_t[:], h_t[:], cb[:])
                if n == 0:
                    ya = scanp.tile([128, TC], BF16, tag=f"y{blk}")
                    nc.vector.tensor_copy(ya[:], p_t[:])
                else:
                    ya = scanp.tile([128, TC], BF16, tag=f"y{blk}")
                    nc.gpsimd.tensor_add(ya[:], yacc[blk][:], p_t[:])
                yacc[blk] = ya

        if DEBUG and c == 0:
            nc.gpsimd.dma_start(dram['dbg_y'][:, :], yacc[0][:])
        # gate + out_proj + residual -> gf (fp16)
        yg_s = []
        for blk in range(NB):
            y2 = tmp.tile([128, TC], BF16, tag="y2")
            nc.vector.scalar_tensor_tensor(y2[:], u_s[blk][:], pc(blk, 32),
                                           yacc[blk][:], AL.mult, AL.add)
            yg = big.tile([128, TC], BF16, tag=f"yg{blk}")
            nc.vector.tensor_mul(yg[:], y2[:], sz_s[blk][:])
            yg_s.append(yg)
        for mb in range(NB):
            p = ps.tile([128, TC], F32, tag="pmm")
            mm_acc(p, w_op, yg_s, slice(mb * 128, (mb + 1) * 128))
            nc.vector.scalar_tensor_tensor(gf_full[mb][:, W], msf_s[mb][:], 0.5,
                                           p[:], AL.mult, AL.add)

    if DEBUG:
        nc.gpsimd.dma_start(dram['dbg_gf'][:, :], gf_full[0][:, 0:TC])
    # ================= 3x3 depthwise conv (fp16, row bands) =================
    BAND = 16  # output rows per band
    for blk in range(NB):
        for b0 in range(0, 64, BAND):
            # padded input band: rows b0-1 .. b0+BAND (BAND+2 rows), 66 cols
            pdrows = BAND + 2
            pd = pp.tile([128, pdrows * 66], F16, tag="pd")
            nc.vector.memset(pd[:], 0.0)
            pdv = pd[:].rearrange("p (h w) -> p h w", h=pdrows)
            r_lo = max(0, b0 - 1)
            r_hi = min(64, b0 + BAND + 1)
            src = gf_full[blk][:, r_lo * 64:r_hi * 64].rearrange(
                "p (h w) -> p h w", w=64)
            nc.vector.tensor_copy(pdv[:, r_lo - (b0 - 1):r_hi - (b0 - 1), 1:65], src)
            acc = pp.tile([128, BAND * 64], F16, tag="dwacc")
            accv = acc[:].rearrange("p (h w) -> p h w", h=BAND)
            nc.vector.tensor_scalar(accv, pdv[:, 0:BAND, 0:64], pc(blk, 20),
                                    pc(blk, 33), AL.mult, AL.add)
            out_f = tmp.tile([128, BAND * 64], F32, tag="dwout")
            for t in range(1, 9):
                ky, kx = t // 3, t % 3
                if t < 8:
                    acc2 = pp.tile([128, BAND * 64], F16, tag="dwacc")
                    dstv = acc2[:].rearrange("p (h w) -> p h w", h=BAND)
                else:
                    acc2 = out_f
                    dstv = acc2[:].rearrange("p (h w) -> p h w", h=BAND)
                nc.vector.scalar_tensor_tensor(
                    dstv, pdv[:, ky:ky + BAND, kx:kx + 64], pc(blk, 20 + t),
                    accv, AL.mult, AL.add)
                acc = acc2
                accv = dstv
            nc.sync.dma_start(
                dram['out'][blk * 128:(blk + 1) * 128, b0 * 64:(b0 + BAND) * 64],
                out_f[:])


_NC_CACHE = None


def kernel(**inputs):
    global _NC_CACHE
    in_maps = []
    for bi in range(4):
        for half in range(2):
            in_maps.append(make_core_inputs(inputs, bi, half))
    if _NC_CACHE is None:
        _NC_CACHE = build_nc()
    res = run_bass_kernel_spmd(_NC_CACHE, in_maps, core_ids=list(range(8)))
    outs = np.zeros((4, DIM, L), np.float32)
    for bi in range(4):
        outs[bi] = res.results[2 * bi]['out'].astype(np.float32) + \
                   res.results[2 * bi + 1]['out'].astype(np.float32)
    return outs.reshape(4, DIM, 64, 64)

